# revision 1
# baseline (speedup 1.0000x reference)
"""Bass/Trainium2 kernel for the BarlowTwins-style cross-entropy loss.

Reference (per batch b of 8):
    logits = z1[b].T @ z2[b] / T            (2048 x 2048, K=256, T=1.0)
    logp   = log_softmax(logits, axis=0)    (softmax over first axis n)
    loss   = -mean_b,m logp[m, m]

Sharding: pure data parallel over the batch axis b -> one batch element per
NeuronCore (8 cores).  Each core computes logitsT[m, n] = sum_s z2[s,m]*z1[s,n]
so the softmax reduction runs along the free axis.

Per 128-row chunk of logitsT, processed as two [128,1024] halves so four
2-bank PSUM tiles keep the PE / DVE / ACT pipeline full:
    nmx_h[m] = -max_{n in h} logitsT[m, n]     (DVE tensor_reduce, negated)
    se_h[m]  = sum_{n in h} exp(logitsT[m,n] + nmx_h[m])   (ACT exp + accum)
Host merge: M = max over halves; se = sum_h se_h * e^(mx_h - M);
            diag[m] = sum_s z1[s,m]*z2[s,m] computed on host in f32 (0.02% of
            the problem FLOPs; the 17 GFLOP logits/softmax runs on device);
            loss = -mean(diag - M - log(se)).

Inputs are converted to fp8e4m3 on the host (quarters the DMA traffic vs
f32 -- the startup is HBM-stream-bound across the 8 cores; the PE runs fp8 at
bf16 rate in normal mode with f32 PSUM accumulation).  The diagonal stays
exact (host f32); only logZ sees the quantization noise: loss error vs the
f32 reference is ~1.3e-3, well inside the 2e-2 gate.
"""

import numpy as np
import ml_dtypes

import concourse.bass as bass
import concourse.tile as tile
from concourse import bacc, mybir
from concourse.bass_utils import run_bass_kernel_spmd

B = 8          # batch (one element per core)
S = 256        # contraction dim
N = 2048       # feature dim (n and m)
P = 128        # SBUF partitions
KC = S // P    # 2 contraction chunks
MC = N // P    # 16 row chunks of logitsT
H = N // 2     # half width (one PSUM tile)

_CACHE = {}


def _build():
    if "nc" in _CACHE:
        return _CACHE["nc"]

    f32 = mybir.dt.float32
    bf16 = mybir.dt.bfloat16
    fp8 = mybir.dt.float8e4

    nc = bacc.Bacc("TRN2", target_bir_lowering=False, debug=False)
    z1 = nc.dram_tensor("z1", [S, N], fp8, kind="ExternalInput").ap()
    z2 = nc.dram_tensor("z2", [S, N], fp8, kind="ExternalInput").ap()
    # col 2m+h of each: negated half maxes / half exp-sums
    mx_d = nc.dram_tensor("mx", [P, 32], f32, kind="ExternalOutput").ap()
    se_d = nc.dram_tensor("se", [P, 32], f32, kind="ExternalOutput").ap()

    z1r = z1.rearrange("(k p) n -> k p n", p=P)
    z2r = z2.rearrange("(k p) n -> k p n", p=P)

    with tile.TileContext(nc) as tc:
        with (
            tc.tile_pool(name="const", bufs=1) as cpool,
            tc.tile_pool(name="zb", bufs=1) as zpool,
            tc.tile_pool(name="psum", bufs=4, space="PSUM") as ppool,
            tc.tile_pool(name="trash", bufs=3) as tpool,
        ):
            # ACT exp-table preload, overlapped with the input DMAs.
            dummy = cpool.tile([1, 1], f32, tag="dummy")
            nc.vector.memset(dummy[:], 0.0)
            nc.scalar.activation(
                dummy[:], dummy[:], mybir.ActivationFunctionType.Exp, bias=0.0
            )

            # Separate staging tiles per writer engine (a single shared
            # tile written by DVE and ACT concurrently measurably slows every
            # engine down -- SBUF region contention).
            omx = cpool.tile([P, 32], f32, tag="omx")
            ose = cpool.tile([P, 32], f32, tag="ose")

            z1b = [
                zpool.tile([P, N], fp8, name=f"z1b{k}", tag=f"z1b{k}")
                for k in range(KC)
            ]
            z2b = [
                zpool.tile([P, N], fp8, name=f"z2b{k}", tag=f"z2b{k}")
                for k in range(KC)
            ]
            # Lean DMA plan: first-needed slices first, spread over both
            # DGE-capable queues so dispatch overlaps.
            # all input DMAs on the sync queue: keeps the Scalar (ACT)
            # queue free of dispatch work so the first exp isn't delayed
            nc.sync.dma_start(z1b[0][:, 0:N], z1r[0][:, 0:N])
            nc.sync.dma_start(z1b[1][:, 0:N], z1r[1][:, 0:N])
            nc.sync.dma_start(z2b[0][:, 0:P], z2r[0][:, 0:P])
            nc.sync.dma_start(z2b[1][:, 0:P], z2r[1][:, 0:P])
            nc.sync.dma_start(z2b[0][:, P:N], z2r[0][:, P:N])
            nc.sync.dma_start(z2b[1][:, P:N], z2r[1][:, P:N])

            # PE warm-up during the input-DMA wait: the HAM clock gate keeps
            # the PE at 1.2 GHz until it has been busy ~3.4us.  A burst of
            # junk matmuls on zeroed SBUF (no DMA dependency) starts that
            # clock early so the first real chunks run at 2.4 GHz.
            warm = cpool.tile([P, 512], bf16, tag="warm")
            nc.vector.memset(warm[:], 0.0)
            jpsum = ppool.tile([P, H], f32, tag="psum")
            for _ in range(6):
                nc.tensor.matmul(
                    jpsum[:, 0:512],
                    lhsT=warm[:, 0:P],
                    rhs=warm[:],
                    start=True,
                    stop=True,
                )

            for m in range(MC):
                ms = slice(m * P, (m + 1) * P)
                for h in range(2):
                    hbase = h * H
                    psum = ppool.tile([P, H], f32, tag="psum")
                    for k in range(KC):
                        for nb in range(2):
                            ns = slice(hbase + nb * 512, hbase + (nb + 1) * 512)
                            nc.tensor.matmul(
                                psum[:, nb * 512 : (nb + 1) * 512],
                                lhsT=z2b[k][:, ms],
                                rhs=z1b[k][:, ns],
                                start=(k == 0),
                                stop=(k == KC - 1),
                            )
                    j = 2 * m + h
                    # negated half-row max -> bias for the exp
                    nc.vector.tensor_reduce(
                        omx[:, j : j + 1],
                        psum[:],
                        axis=mybir.AxisListType.X,
                        op=mybir.AluOpType.max,
                        negate=True,
                    )
                    # exp(logitsT - halfmax), accumulated along the half
                    # row; the exp values themselves are discarded (rotating
                    # SBUF trash tiles -- a single tile would WAW-serialize
                    # consecutive ACT ops, writing PSUM in place would
                    # contend with the matmul writes).
                    trash = tpool.tile([P, H], bf16, tag="trash")
                    nc.scalar.activation(
                        trash[:],
                        psum[:],
                        mybir.ActivationFunctionType.Exp,
                        bias=omx[:, j : j + 1],
                        scale=1.0,
                        accum_out=ose[:, j : j + 1],
                    )
                if m == 13:
                    # Overlap most of the output DMA with the last chunks.
                    nc.sync.dma_start(mx_d[:, 0:28], omx[:, 0:28])
                    nc.sync.dma_start(se_d[:, 0:28], ose[:, 0:28])

            nc.sync.dma_start(mx_d[:, 28:32], omx[:, 28:32])
            nc.scalar.dma_start(se_d[:, 28:32], ose[:, 28:32])

    nc.compile()
    _CACHE["nc"] = nc
    return nc


def _run(z1, z2, **spmd_kwargs):
    """Shard over batch, run on 8 cores, return (loss, BassKernelResults)."""
    nc = _build()
    z1 = np.ascontiguousarray(z1, dtype=np.float32)
    z2 = np.ascontiguousarray(z2, dtype=np.float32)
    in_maps = [
        {
            "z1": np.ascontiguousarray(z1[b].astype(ml_dtypes.float8_e4m3)),
            "z2": np.ascontiguousarray(z2[b].astype(ml_dtypes.float8_e4m3)),
        }
        for b in range(B)
    ]
    res = run_bass_kernel_spmd(nc, in_maps, core_ids=list(range(B)), **spmd_kwargs)

    # diag[b, m] = sum_s z1[b,s,m] * z2[b,s,m] in f32 on host (tiny vs the
    # 17 GFLOP device part).
    dg = np.einsum("bsm,bsm->bm", z1, z2, dtype=np.float64)

    total = 0.0
    for b in range(B):
        nmx = res.results[b]["mx"].astype(np.float64)  # negated piece maxes
        se2 = res.results[b]["se"].astype(np.float64)
        ma = -nmx[:, 0::2]                             # [P, MC] left-half max
        mb = -nmx[:, 1::2]
        sea = se2[:, 0::2]
        seb = se2[:, 1::2]
        M = np.maximum(ma, mb)
        se = sea * np.exp(ma - M) + seb * np.exp(mb - M)
        logZ = (M + np.log(se)).T.reshape(N)           # row index = m*P + p
        total += np.sum(dg[b] - logZ)
    loss = -total / (B * N)
    return np.asarray(loss, dtype=np.float32), res


def kernel(z1, z2):
    loss, _ = _run(z1, z2)
    return loss



# revision 7
# speedup vs baseline: 1.1256x; 1.1256x over previous
"""Bass/Trainium2 kernel for the BarlowTwins-style cross-entropy loss.

Reference (per batch b of 8):
    logits = z1[b].T @ z2[b] / T            (2048 x 2048, K=256, T=1.0)
    logp   = log_softmax(logits, axis=0)
    loss   = -mean_b,m logp[m, m]

Sharding: pure data parallel over batch b -> one element per NeuronCore.
Each core computes logitsT[m, n] = sum_s z2[s,m]*z1[s,n] in 16 row-blocks
of [128 m x 2048 n] so the softmax reduction runs along the free axis.

Per-row shift: logZ[m] needs exp(x - c_m) with c_m within ~85 nats of the
row max.  c_m = max(diag[m], 60) + 40 with diag computed on host (it is
needed for the loss anyway and diag is an entry of row m, so c_m <= rowmax
+ 80 always holds -> no underflow).  Rows whose max exceeds c_m + ~85
overflow to inf/NaN on device; the host detects those (~1.8% on this data)
and recomputes their logZ exactly (256x2048 dot each -- negligible).

Engines (all fp8 DoubleRow matmuls, K=256 in one MM):
  - PE:  4 x [128x512] DoubleRow MMs per block into a [128,2048] PSUM tile.
  - ACT blocks: one 2048-wide exp with bias=-c_m and accum_out -> se[m].
  - DVE blocks (Schraudolph exp): uint16 bits = round(x*184.665 + (16256 -
    184.665*c_m)), saturating both ways (verified on HW: negative -> 0 ->
    bf16 +0.0, huge -> 65535 -> NaN so overflow stays detectable).  The
    uint16 tile bitcast to bf16 *is* e^(x-c_m) to ~3%; a fused
    tensor_tensor_reduce folds halves and row-sums in one pass.
    The ~1% Schraudolph bias on a third of the blocks shifts the loss by
    ~5e-3 relative -- the gate is 2e-2.

Host merge: logZ = c_m + log(se); loss = -mean(diag - logZ).
"""

import numpy as np
import ml_dtypes

import concourse.bass as bass
import concourse.tile as tile
from concourse import bacc, mybir
from concourse.bass_utils import run_bass_kernel_spmd

B = 8          # batch (one element per core)
S = 256        # contraction dim
N = 2048       # feature dim (n and m)
P = 128        # SBUF partitions
NBLK = N // P  # 16 row blocks
NQ = 4         # 512-wide psum quarters per block

A_SCH = 128.0 / float(np.log(2.0))       # Schraudolph scale (bf16 bits/nat)
DVE_BLOCKS = (2, 5, 8, 11, 14)             # blocks whose exp runs on the DVE

_CACHE = {}


def _build(dve_blocks=DVE_BLOCKS, use_ttr=True):
    key = ("nc", tuple(sorted(dve_blocks)), use_ttr)
    if key in _CACHE:
        return _CACHE[key]

    f32 = mybir.dt.float32
    bf16 = mybir.dt.bfloat16
    u16 = mybir.dt.uint16
    fp8 = mybir.dt.float8e4

    nc = bacc.Bacc("TRN2", target_bir_lowering=False, debug=False)
    # z1: moving operand, [128, q, ktile, 512] (q-major so rhs slices are
    # contiguous); z2: weights, [128, blk, ktile, 128].
    z1 = nc.dram_tensor("z1", [P, NQ, 2, 512], fp8, kind="ExternalInput").ap()
    z2 = nc.dram_tensor("z2", [P, NBLK, 2, P], fp8, kind="ExternalInput").ap()
    cmx = nc.dram_tensor("cmx", [P, NBLK], f32, kind="ExternalInput").ap()
    bsc = nc.dram_tensor("bsc", [P, NBLK], f32, kind="ExternalInput").ap()
    sea_d = nc.dram_tensor("sea", [P, NBLK], f32, kind="ExternalOutput").ap()
    sed_d = nc.dram_tensor("sed", [P, NBLK], f32, kind="ExternalOutput").ap()

    with tile.TileContext(nc) as tc:
        with (
            tc.tile_pool(name="const", bufs=1) as cpool,
            tc.tile_pool(name="zb", bufs=1) as zpool,
            tc.tile_pool(name="psum", bufs=2, space="PSUM") as ppool,
            tc.tile_pool(name="trash", bufs=2) as tpool,
            tc.tile_pool(name="sch", bufs=2) as spool,
        ):
            # ACT exp-table preload, overlapped with the input DMAs.
            dummy = cpool.tile([1, 1], f32, tag="dummy")
            nc.vector.memset(dummy[:], 0.0)
            nc.scalar.activation(
                dummy[:], dummy[:], mybir.ActivationFunctionType.Exp, bias=0.0
            )

            z1t = zpool.tile([P, NQ, 2, 512], fp8, tag="z1t")
            z2t = zpool.tile([P, NBLK, 2, P], fp8, tag="z2t")
            cmt = cpool.tile([P, NBLK], f32, tag="cmt")
            bst = cpool.tile([P, NBLK], f32, tag="bst")
            # First-needed slices first, all on the sync queue.
            nc.sync.dma_start(z2t[:, 0:2], z2[:, 0:2])
            nc.sync.dma_start(z1t[:, 0], z1[:, 0])
            nc.sync.dma_start(cmt[:], cmx)
            nc.sync.dma_start(bst[:], bsc)
            nc.sync.dma_start(z1t[:, 1], z1[:, 1])
            nc.sync.dma_start(z1t[:, 2], z1[:, 2])
            nc.sync.dma_start(z1t[:, 3], z1[:, 3])
            nc.sync.dma_start(z2t[:, 2:NBLK], z2[:, 2:NBLK])

            # Outputs; each engine has its own staging tile.
            sea = cpool.tile([P, NBLK], f32, tag="sea")
            sed = cpool.tile([P, NBLK], f32, tag="sed")
            nc.scalar.memzero(sea[:])
            nc.vector.memset(sed[:], 0.0)

            # PE warm-up during the DMA wait: junk matmuls on zeroed SBUF
            # start the HAM activity clock so real MMs run at 2.4 GHz.
            warm = cpool.tile([P, 512], bf16, tag="warm")
            nc.vector.memset(warm[:], 0.0)
            jp = ppool.tile([P, N], f32, tag="ps")
            for _ in range(6):
                nc.tensor.matmul(
                    jp[:, 0:512],
                    lhsT=warm[:, 0:P],
                    rhs=warm[:],
                    start=True,
                    stop=True,
                )

            for blk in range(NBLK):
                ps = ppool.tile([P, N], f32, tag="ps")
                for q in range(NQ):
                    nc.tensor.matmul(
                        ps[:, q * 512 : (q + 1) * 512],
                        lhsT=z2t[:, blk],
                        rhs=z1t[:, q],
                        start=True,
                        stop=True,
                        perf_mode=mybir.MatmulPerfMode.DoubleRow,
                    )
                if blk in dve_blocks:
                    # Schraudolph: uint16 bits of bf16(e^(x - c_m)).
                    ut = spool.tile([P, N], u16, tag="ut")
                    nc.vector.tensor_scalar(
                        ut[:],
                        ps[:],
                        A_SCH,
                        bst[:, blk : blk + 1],
                        op0=mybir.AluOpType.mult,
                        op1=mybir.AluOpType.add,
                    )
                    vb = ut[:].bitcast(bf16)
                    if use_ttr:
                        # bf16 pairwise fold (2x_1P mode) then a shorter
                        # 1x reduce -- ~2.2x cheaper than reducing 2048.
                        fold = spool.tile([P, N // 2], bf16, tag="fold")
                        nc.vector.tensor_tensor(
                            fold[:],
                            vb[:, 0 : N // 2],
                            vb[:, N // 2 : N],
                            op=mybir.AluOpType.add,
                        )
                        nc.vector.tensor_reduce(
                            sed[:, blk : blk + 1],
                            fold[:],
                            axis=mybir.AxisListType.X,
                            op=mybir.AluOpType.add,
                        )
                    else:
                        nc.vector.tensor_reduce(
                            sed[:, blk : blk + 1],
                            vb[:],
                            axis=mybir.AxisListType.X,
                            op=mybir.AluOpType.add,
                        )
                else:
                    trash = tpool.tile([P, N], bf16, tag="trash")
                    nc.scalar.activation(
                        trash[:],
                        ps[:],
                        mybir.ActivationFunctionType.Exp,
                        bias=cmt[:, blk : blk + 1],
                        scale=1.0,
                        accum_out=sea[:, blk : blk + 1],
                    )

            nc.scalar.dma_start(sea_d, sea[:])
            nc.sync.dma_start(sed_d, sed[:])

    nc.compile()
    _CACHE[key] = nc
    return nc


def _prep(z1, z2):
    """Host-side packing: fp8 + DoubleRow interleave + per-row bias."""
    z1 = np.ascontiguousarray(z1, dtype=np.float32)
    z2 = np.ascontiguousarray(z2, dtype=np.float32)
    dg64 = np.einsum("bsm,bsm->bm", z1, z2, dtype=np.float64)
    c = (np.maximum(dg64.astype(np.float32), 60.0) + 40.0).astype(np.float32)

    z1f = z1.astype(ml_dtypes.float8_e4m3)
    z2f = z2.astype(ml_dtypes.float8_e4m3)

    in_maps = []
    for b in range(B):
        # [s, n] -> [p, q, ktile, 512] with s = ktile*128 + p
        z1b = np.ascontiguousarray(
            z1f[b].reshape(2, P, NQ, 512).transpose(1, 2, 0, 3)
        )
        z2b = np.ascontiguousarray(
            z2f[b].reshape(2, P, NBLK, P).transpose(1, 2, 0, 3)
        )
        cb = c[b].reshape(NBLK, P).T  # [p, blk], m = blk*128 + p
        cmx = np.ascontiguousarray(-cb)
        bsc = np.ascontiguousarray(16256.0 - A_SCH * cb).astype(np.float32)
        in_maps.append(
            {"z1": z1b, "z2": z2b, "cmx": cmx, "bsc": bsc.astype(np.float32)}
        )
    return z1, z2, dg64, c, in_maps


def _run(z1, z2, dve_blocks=DVE_BLOCKS, use_ttr=True, **spmd_kwargs):
    nc = _build(dve_blocks, use_ttr)
    z1, z2, dg64, c, in_maps = _prep(z1, z2)
    res = run_bass_kernel_spmd(nc, in_maps, core_ids=list(range(B)), **spmd_kwargs)

    dve = set(dve_blocks)
    total = 0.0
    npatch = 0
    for b in range(B):
        sea = res.results[b]["sea"]  # [p, blk]
        sed = res.results[b]["sed"]
        se = np.where(
            np.isin(np.arange(NBLK)[None, :], list(dve)), sed, sea
        )  # [p, blk]
        se_m = se.T.reshape(N)  # m = blk*128 + p
        cb = c[b].astype(np.float64)
        bad = ~np.isfinite(se_m) | (se_m <= 0.0)
        with np.errstate(divide="ignore", invalid="ignore"):
            logZ = cb + np.log(se_m.astype(np.float64))
        if bad.any():
            idx = np.where(bad)[0]
            npatch += len(idx)
            rows = z2[b][:, idx].T.astype(np.float64) @ z1[b].astype(np.float64)
            m0 = rows.max(axis=1)
            logZ[idx] = m0 + np.log(np.exp(rows - m0[:, None]).sum(axis=1))
        total += (dg64[b] - logZ).sum()
    loss = -total / (B * N)
    return np.asarray(loss, dtype=np.float32), res


def kernel(z1, z2):
    loss, _ = _run(z1, z2)
    return loss
